# revision 1
# baseline (speedup 1.0000x reference)
"""Cubic B-spline FFD 3D upsampling kernel for Trainium2 (8 NeuronCores).

Reference computation: v [4,3,44,52,44] f32 -> out [4,3,160,192,160] f32 via three
separable stride-4 transposed convs (cubic B-spline, kernel len 15) + crop [4:4+D].

Sharding: output z axis (160) split into 8 chunks of 20; core c consumes input
z-planes [5c, 5c+8) (no halo exchange needed) and writes its own [12,20,192,160]
block. Host slices/concats only (no transposes).

Per-core pipeline (all shapes [partition, free]):
  L0 [128=(g2*64 + yi52), (b6, zi8, xi44)]      bf16, DMA-in
  z-pass on DVE: fused scalar_tensor_tensor MACs (polyphase, zo=4k+r)
  L1 [128, (b6, zo20, xi-pad64)]                bf16
  y-pass on PE:  out[xi,yo] = L1[yi,xi].T @ Wy[yi,yo]  per (g,b,zo), bf16
  L2b [128=(g2*64 + xi44pad), (zo20, yo192)]    bf16  (per b)
  x-pass on PE:  out[m,xo] = L2b[xi, m-chunk].T @ Wx[xi,xo],  m=(zo,yo) flat
  PSUM f32 -> SBUF f32 -> DMA out, xo contiguous (final layout, no transpose)
"""

import numpy as np

N_CORES = 8
ZIN, YIN, XIN = 44, 52, 44
ZOUT, YOUT, XOUT = 160, 192, 160
BC = 12  # batch*channels
ZSH = ZOUT // N_CORES      # 20 output z per core
ZISH = 8                   # input z planes per core


def _bspline_kernel():
    x = (np.arange(15) - 7) / 4.0
    t = np.abs(x)
    return np.where(
        t < 1.0, 2.0 / 3.0 + (0.5 * t - 1.0) * t**2,
        np.where(t < 2.0, ((2.0 - t) ** 3) / 6.0, 0.0)
    ).astype(np.float32)


_W = _bspline_kernel()


def _exp_mat(n_in, n_out):
    """M[i, o] = weight of control point i on (post-crop) output o."""
    M = np.zeros((n_in, n_out), dtype=np.float32)
    for o in range(n_out):
        ilo = int(np.ceil((o - 3) / 4))
        ihi = (o + 11) // 4
        for i in range(max(ilo, 0), min(ihi, n_in - 1) + 1):
            n = 4 * i - o + 3
            if 0 <= n < 15:
                M[i, o] = _W[n]
    return M


def _ztaps():
    """Per phase r: list of (tap t, weight) with input plane = k + t for zo=4k+r."""
    out = []
    for r in range(4):
        taps = []
        for t in range(4):
            n = 4 * t + 3 - r
            if 0 <= n < 15:
                taps.append((t, float(_W[n])))
        out.append(taps)
    return out


_NC_CACHE = {}


def _build_nc():
    import concourse.bacc as bacc
    import concourse.mybir as mybir
    from concourse.tile import TileContext

    FP32 = mybir.dt.float32
    BF16 = mybir.dt.bfloat16
    MULT = mybir.AluOpType.mult
    ADD = mybir.AluOpType.add

    nc = bacc.Bacc()
    v = nc.declare_dram_parameter("v", [BC, ZISH, YIN, XIN], BF16, isOutput=False)
    wy = nc.declare_dram_parameter("wy", [128, YOUT], BF16, isOutput=False)
    wx = nc.declare_dram_parameter("wx", [128, XOUT], BF16, isOutput=False)
    out = nc.declare_dram_parameter(
        "out", [BC, ZSH, YOUT, XOUT], FP32, isOutput=True
    )
    outflat = out.rearrange("b z y x -> (b z y) x")  # [46080, 160]

    ztaps = _ztaps()
    XP = 64  # xi padded to 64 in L1 so two (g) matmuls col-tile at bases {0, 64}

    with TileContext(nc) as tc:
        with (
            tc.tile_pool(name="const", bufs=1) as cpool,
            tc.tile_pool(name="io", bufs=1) as iopool,
            tc.tile_pool(name="l2", bufs=4) as l2pool,
            tc.tile_pool(name="stage", bufs=12) as stpool,
            tc.tile_pool(name="psy", bufs=4, space="PSUM") as psy,
            tc.tile_pool(name="psx", bufs=4, space="PSUM") as psx,
        ):
            wyt = cpool.tile([128, YOUT], BF16)
            nc.sync.dma_start(out=wyt[:, :], in_=wy[:, :])
            wxt = cpool.tile([128, XOUT], BF16)
            nc.sync.dma_start(out=wxt[:, :], in_=wx[:, :])

            L0 = iopool.tile([128, 6 * ZISH * XIN], BF16)   # (b, zi, xi)
            L1 = iopool.tile([128, 6 * ZSH * XP], BF16)     # (b, zo, xi-pad)

            for g in range(2):
                nc.sync.dma_start(
                    out=L0[64 * g:64 * g + YIN, :]
                    .rearrange("p (b z x) -> p b z x", b=6, z=ZISH),
                    in_=v[6 * g:6 * g + 6].rearrange("b z y x -> y b z x"),
                )

            L0v = L0.rearrange("p (b z x) -> p b z x", b=6, z=ZISH)
            # zo = 4k + r  (k-major, r-minor view)
            L1r = L1.rearrange("p (b k r x) -> p b k r x", b=6, k=5, r=4)
            L1z = L1.rearrange("p (b z x) -> p b z x", b=6, z=ZSH)

            # ---- z-pass (DVE fused MACs), all b at once, 15 instructions ----
            for g in range(2):
                lo, hi = 64 * g, 64 * g + YIN
                for r in range(4):
                    dst = L1r[lo:hi, :, :, r, 0:XIN]
                    t0, w0 = ztaps[r][0]
                    nc.vector.tensor_scalar_mul(dst, L0v[lo:hi, :, t0:t0 + 5, :], w0)
                    for t, w in ztaps[r][1:]:
                        nc.vector.scalar_tensor_tensor(
                            out=dst, in0=L0v[lo:hi, :, t:t + 5, :], scalar=w,
                            in1=dst, op0=MULT, op1=ADD,
                        )

            # ---- per-b: y-pass (PE) -> L2b, then x-pass (PE) -> DMA out ----
            ncopy = 0
            for b in range(6):
                L2b = l2pool.tile([128, ZSH * YOUT], BF16)
                for zp in range(ZSH // 2):
                    py = psy.tile([128, 2 * YOUT], FP32)
                    for i in range(2):
                        zo = 2 * zp + i
                        for g in range(2):
                            nc.tensor.matmul(
                                py[64 * g:64 * g + XP, i * YOUT:(i + 1) * YOUT],
                                lhsT=L1z[64 * g:64 * g + YIN, b, zo, :],
                                rhs=wyt[64 * g:64 * g + YIN, :],
                                start=True, stop=True,
                            )
                    dst = L2b[:, zp * 2 * YOUT:(zp + 1) * 2 * YOUT]
                    if ncopy % 2 == 0:
                        nc.vector.tensor_copy(out=dst, in_=py[:, :])
                    else:
                        nc.scalar.copy(dst, py[:, :])
                    ncopy += 1

                for g in range(2):
                    for cg in range(10):
                        px = psx.tile([128, 3 * XOUT], FP32)
                        for j in range(3):
                            c = cg * 3 + j
                            nc.tensor.matmul(
                                px[:, j * XOUT:(j + 1) * XOUT],
                                lhsT=L2b[64 * g:64 * g + XIN,
                                         c * 128:(c + 1) * 128],
                                rhs=wxt[64 * g:64 * g + XIN, :],
                                start=True, stop=True,
                            )
                        st = stpool.tile([128, 3 * XOUT], FP32)
                        if ncopy % 2 == 0:
                            nc.vector.tensor_copy(out=st[:, :], in_=px[:, :])
                        else:
                            nc.scalar.copy(st[:, :], px[:, :])
                        ncopy += 1
                        base = (g * 6 + b) * ZSH * YOUT + cg * 384
                        nc.sync.dma_start(
                            out=outflat[base:base + 384, :].rearrange(
                                "(j p) x -> p j x", p=128),
                            in_=st.rearrange("p (j x) -> p j x", j=3),
                        )
    nc.compile()
    return nc


def _get_nc():
    if "nc" not in _NC_CACHE:
        _NC_CACHE["nc"] = _build_nc()
    return _NC_CACHE["nc"]


def kernel(v):
    import ml_dtypes
    from concourse.bass_utils import run_bass_kernel_spmd

    bf16 = ml_dtypes.bfloat16
    v = np.asarray(v).astype(np.float32).reshape(BC, ZIN, YIN, XIN)

    wy128 = np.zeros((128, YOUT), dtype=np.float32)
    wy128[0:YIN_Y] = _exp_mat(YIN_Y, YOUT)
    wy128[64:64 + YIN_Y] = wy128[0:YIN_Y]
    wx128 = np.zeros((128, XOUT), dtype=np.float32)
    wx128[0:XIN] = _exp_mat(XIN, XOUT)
    wx128[64:64 + XIN] = wx128[0:XIN]
    wy_b = wy128.astype(bf16)
    wx_b = wx128.astype(bf16)

    in_maps = []
    for c in range(N_CORES):
        slab = np.ascontiguousarray(v[:, 5 * c:5 * c + ZISH]).astype(bf16)
        in_maps.append({"v": slab, "wy": wy_b, "wx": wx_b})

    nc = _get_nc()
    res = run_bass_kernel_spmd(nc, in_maps, core_ids=list(range(N_CORES)))

    out = np.empty((BC, ZOUT, YOUT, XOUT), dtype=np.float32)
    for c in range(N_CORES):
        out[:, ZSH * c:ZSH * (c + 1)] = res.results[c]["out"]
    return out.reshape(4, 3, ZOUT, YOUT, XOUT)


YIN_Y = YIN  # y-axis input size (52)



# revision 3
# speedup vs baseline: 1.3880x; 1.3880x over previous
"""Cubic B-spline FFD 3D upsampling kernel for Trainium2 (8 NeuronCores).

Reference: v [4,3,44,52,44] f32 -> out [4,3,160,192,160] f32 via three separable
stride-4 transposed convs (cubic B-spline, 15 taps) + crop.

Sharding: output z (160) split into 8 slabs of 20; core c reads input z-planes
[5c, 5c+8) (no halo) and writes out[:, :, 20c:20c+20].

Per-core pipeline (fp16 data, f32 PSUM):
  L0 [128=(g*64 + y52), (b6, zi8, xi44)]   one DMA in, host-packed layout
  z-pass DVE: 15 fused MACs over both g halves at once (partitions 0:116)
  L1 [128, (b6, zo20, xi44)]
  y-pass PE:  py[64g+xi44, zo2*192] = L1[y, b, zo, xi].T @ Wy   (4 mm / 2 zo)
  L2 [128, m=(zo20, yo192)=3840] per b  (copy downcast fp16, DVE/Act split)
  x-pass PE:  px[128=m-chunk(stride 30), xo160] = L2[xi, m].T @ Wx
  st [128, (r30, xo160)] fp16: partition p holds DRAM rows 30p..30p+29
  DMA out per (b,g): 9600B descriptors -> out fp16 [12, 20, 192, 160]
Host: cast fp16 -> f32, stack z-slabs.
"""

import numpy as np

N_CORES = 8
ZIN, YIN, XIN = 44, 52, 44
ZOUT, YOUT, XOUT = 160, 192, 160
BC = 12
ZSH = ZOUT // N_CORES      # 20 output z per core
ZISH = 8                   # input z planes per core
M = ZSH * YOUT             # 3840 rows per (b, g) block
NJ = 30                    # rows per partition in x-pass/st/DMA


def _bspline_kernel():
    x = (np.arange(15) - 7) / 4.0
    t = np.abs(x)
    return np.where(
        t < 1.0, 2.0 / 3.0 + (0.5 * t - 1.0) * t**2,
        np.where(t < 2.0, ((2.0 - t) ** 3) / 6.0, 0.0)
    ).astype(np.float32)


_W = _bspline_kernel()


def _exp_mat(n_in, n_out):
    """M[i, o] = weight of control point i on (post-crop) output o."""
    Mm = np.zeros((n_in, n_out), dtype=np.float32)
    for o in range(n_out):
        ilo = int(np.ceil((o - 3) / 4))
        ihi = (o + 11) // 4
        for i in range(max(ilo, 0), min(ihi, n_in - 1) + 1):
            n = 4 * i - o + 3
            if 0 <= n < 15:
                Mm[i, o] = _W[n]
    return Mm


def _ztaps():
    """Per phase r: list of (tap t, weight) with input plane = k + t for zo=4k+r."""
    out = []
    for r in range(4):
        taps = []
        for t in range(4):
            n = 4 * t + 3 - r
            if 0 <= n < 15:
                taps.append((t, float(_W[n])))
        out.append(taps)
    return out


_NC_CACHE = {}


def _build_nc():
    import concourse.bacc as bacc
    import concourse.mybir as mybir
    from concourse.tile import TileContext

    FP32 = mybir.dt.float32
    FP16 = mybir.dt.float16
    MULT = mybir.AluOpType.mult
    ADD = mybir.AluOpType.add

    nc = bacc.Bacc()
    v = nc.declare_dram_parameter("v", [128, 6 * ZISH * XIN], FP16, isOutput=False)
    wy = nc.declare_dram_parameter("wy", [128, YOUT], FP16, isOutput=False)
    wx = nc.declare_dram_parameter("wx", [128, XOUT], FP16, isOutput=False)
    out = nc.declare_dram_parameter("out", [BC, ZSH, YOUT, XOUT], FP16, isOutput=True)
    outflat = out.rearrange("b z y x -> (b z y) x")  # [46080, 160]

    ztaps = _ztaps()

    with TileContext(nc) as tc:
        with (
            tc.tile_pool(name="const", bufs=1) as cpool,
            tc.tile_pool(name="io", bufs=1) as iopool,
            tc.tile_pool(name="l2", bufs=3) as l2pool,
            tc.tile_pool(name="stage", bufs=3) as stpool,
            tc.tile_pool(name="psy", bufs=2, space="PSUM") as psy,
            tc.tile_pool(name="psx", bufs=3, space="PSUM") as psx,
        ):
            wyt = cpool.tile([128, YOUT], FP16)
            nc.sync.dma_start(out=wyt[:, :], in_=wy[:, :])
            wxt = cpool.tile([128, XOUT], FP16)
            nc.sync.dma_start(out=wxt[:, :], in_=wx[:, :])

            L0 = iopool.tile([128, 6 * ZISH * XIN], FP16)
            nc.sync.dma_start(out=L0[:, :], in_=v[:, :])
            L0v = L0.rearrange("p (b z x) -> p b z x", b=6, z=ZISH)

            L1 = iopool.tile([128, 6 * ZSH * XIN], FP16)
            L1r = L1.rearrange("p (b k r x) -> p b k r x", b=6, k=5, r=4)
            L1z = L1.rearrange("p (b z x) -> p b z x", b=6, z=ZSH)

            # ---- z-pass on DVE: both g halves at once, 15 instructions ----
            for r in range(4):
                dst = L1r[0:116, :, :, r, :]
                t0, w0 = ztaps[r][0]
                nc.vector.tensor_scalar_mul(dst, L0v[0:116, :, t0:t0 + 5, :], w0)
                for t, w in ztaps[r][1:]:
                    nc.vector.scalar_tensor_tensor(
                        out=dst, in0=L0v[0:116, :, t:t + 5, :], scalar=w,
                        in1=dst, op0=MULT, op1=ADD,
                    )

            # copy-engine chooser: static split tuned for DVE(z)+copies vs Act
            def cp(engine, dst, src):
                if engine == "v":
                    nc.vector.tensor_copy(out=dst, in_=src)
                else:
                    nc.scalar.copy(dst, src)

            def emit_y(b, L2b):
                # Act takes early-b copies (DVE busy with z); DVE later ones
                eng = "a" if b < 2 else "v"
                for zp in range(ZSH // 2):
                    py = psy.tile([128, 2 * YOUT], FP32)
                    for i in range(2):
                        zo = 2 * zp + i
                        for g in range(2):
                            nc.tensor.matmul(
                                py[64 * g:64 * g + XIN, i * YOUT:(i + 1) * YOUT],
                                lhsT=L1z[64 * g:64 * g + YIN, b, zo, :],
                                rhs=wyt[64 * g:64 * g + YIN, :],
                                start=True, stop=True,
                            )
                    cp(eng, L2b[0:108, zp * 2 * YOUT:(zp + 1) * 2 * YOUT],
                       py[0:108, :])

            def emit_x(b, L2b):
                L2v = L2b.rearrange("p (m j) -> p m j", j=NJ)
                for g in range(2):
                    st = stpool.tile([128, NJ * XOUT], FP16)
                    stv = st.rearrange("p (s t c) -> p s t c", s=5, t=2)
                    for s in range(5):
                        px = psx.tile([128, 1024], FP32)
                        pxv = px.rearrange("p (t c) -> p t c", t=2)
                        for t in range(2):
                            for u in range(3):
                                j = 6 * s + 3 * t + u
                                nc.tensor.matmul(
                                    px[:, 512 * t + 160 * u:512 * t + 160 * (u + 1)],
                                    lhsT=L2v[64 * g:64 * g + XIN, :, j],
                                    rhs=wxt[64 * g:64 * g + XIN, :],
                                    start=True, stop=True,
                                )
                        # x-copies: mostly Act; DVE takes a few late ones
                        eng = "v" if (b >= 4 and s >= 3) else "a"
                        cp(eng, stv[:, s, :, :], pxv[:, :, 0:480])
                    base = (g * 6 + b) * M
                    nc.sync.dma_start(
                        out=outflat[base:base + M, :]
                        .rearrange("(p r) x -> p (r x)", p=128),
                        in_=st[:, :],
                    )

            L2 = [None] * 6
            L2[0] = l2pool.tile([128, M], FP16, name='l2_0')
            emit_y(0, L2[0])
            for b in range(6):
                if b < 5:
                    L2[b + 1] = l2pool.tile([128, M], FP16, name=f'l2_{b+1}')
                    emit_y(b + 1, L2[b + 1])
                emit_x(b, L2[b])
    nc.compile()
    return nc


def _get_nc():
    if "nc" not in _NC_CACHE:
        _NC_CACHE["nc"] = _build_nc()
    return _NC_CACHE["nc"]


def _host_inputs(v):
    """Per-core input slabs + weight tiles."""
    v = np.asarray(v).astype(np.float32).reshape(BC, ZIN, YIN, XIN)

    wy128 = np.zeros((128, YOUT), dtype=np.float32)
    wy128[0:YIN] = _exp_mat(YIN, YOUT)
    wy128[64:64 + YIN] = wy128[0:YIN]
    wx128 = np.zeros((128, XOUT), dtype=np.float32)
    wx128[0:XIN] = _exp_mat(XIN, XOUT)
    wx128[64:64 + XIN] = wx128[0:XIN]
    wy_h = wy128.astype(np.float16)
    wx_h = wx128.astype(np.float16)

    in_maps = []
    for c in range(N_CORES):
        slab = np.zeros((128, 6, ZISH, XIN), dtype=np.float16)
        vv = v[:, 5 * c:5 * c + ZISH]                  # [12, 8, 52, 44]
        slab[0:YIN] = vv[0:6].transpose(2, 0, 1, 3)    # y b z x
        slab[64:64 + YIN] = vv[6:12].transpose(2, 0, 1, 3)
        in_maps.append({
            "v": slab.reshape(128, 6 * ZISH * XIN),
            "wy": wy_h, "wx": wx_h,
        })
    return in_maps


def kernel(v):
    from concourse.bass_utils import run_bass_kernel_spmd

    in_maps = _host_inputs(v)
    nc = _get_nc()
    res = run_bass_kernel_spmd(nc, in_maps, core_ids=list(range(N_CORES)))

    out = np.empty((BC, ZOUT, YOUT, XOUT), dtype=np.float32)
    for c in range(N_CORES):
        out[:, ZSH * c:ZSH * (c + 1)] = np.asarray(
            res.results[c]["out"]).astype(np.float32)
    return out.reshape(4, 3, ZOUT, YOUT, XOUT)


# revision 5
# speedup vs baseline: 1.4577x; 1.0502x over previous
"""Cubic B-spline FFD 3D upsampling kernel for Trainium2 (8 NeuronCores).

Reference: v [4,3,44,52,44] f32 -> out [4,3,160,192,160] f32 via three separable
stride-4 transposed convs (cubic B-spline, 15 taps) + crop.

Sharding: output z (160) split into 8 slabs of 20; core c reads input z-planes
[5c, 5c+8) (no halo) and writes out[:, :, 20c:20c+20].

Per-core pipeline (fp16 data, f32 PSUM):
  L0 [128=(g*64 + y52), (b6, zi8, xi44)]   one DMA in, host-packed layout
  z-pass DVE: 15 fused MACs over both g halves at once (partitions 0:116)
  L1 [128, (b6, zo20, xi44)]
  y-pass PE:  py[64g+xi44, zo2*192] = L1[y, b, zo, xi].T @ Wy   (4 mm / 2 zo)
  L2 [128, m=(zo20, yo192)=3840] per b  (copy downcast fp16, DVE/Act split)
  x-pass PE:  px[128=m-chunk(stride 30), xo160] = L2[xi, m].T @ Wx
  st [128, (r30, xo160)] fp16: partition p holds DRAM rows 30p..30p+29
  DMA out per (b,g): 9600B descriptors -> out fp16 [12, 20, 192, 160]
Host: cast fp16 -> f32, stack z-slabs.
"""

import numpy as np

N_CORES = 8
ZIN, YIN, XIN = 44, 52, 44
ZOUT, YOUT, XOUT = 160, 192, 160
BC = 12
ZSH = ZOUT // N_CORES      # 20 output z per core
ZISH = 8                   # input z planes per core
M = ZSH * YOUT             # 3840 rows per (b, g) block
NJ = 30                    # rows per partition in x-pass/st/DMA


def _bspline_kernel():
    x = (np.arange(15) - 7) / 4.0
    t = np.abs(x)
    return np.where(
        t < 1.0, 2.0 / 3.0 + (0.5 * t - 1.0) * t**2,
        np.where(t < 2.0, ((2.0 - t) ** 3) / 6.0, 0.0)
    ).astype(np.float32)


_W = _bspline_kernel()


def _exp_mat(n_in, n_out):
    """M[i, o] = weight of control point i on (post-crop) output o."""
    Mm = np.zeros((n_in, n_out), dtype=np.float32)
    for o in range(n_out):
        ilo = int(np.ceil((o - 3) / 4))
        ihi = (o + 11) // 4
        for i in range(max(ilo, 0), min(ihi, n_in - 1) + 1):
            n = 4 * i - o + 3
            if 0 <= n < 15:
                Mm[i, o] = _W[n]
    return Mm


def _ztaps():
    """Per phase r: list of (tap t, weight) with input plane = k + t for zo=4k+r."""
    out = []
    for r in range(4):
        taps = []
        for t in range(4):
            n = 4 * t + 3 - r
            if 0 <= n < 15:
                taps.append((t, float(_W[n])))
        out.append(taps)
    return out


_NC_CACHE = {}


def _build_nc():
    import concourse.bacc as bacc
    import concourse.mybir as mybir
    from concourse.tile import TileContext

    FP32 = mybir.dt.float32
    FP16 = mybir.dt.float16
    MULT = mybir.AluOpType.mult
    ADD = mybir.AluOpType.add

    nc = bacc.Bacc()
    v = nc.declare_dram_parameter("v", [128, 6 * ZISH * XIN], FP16, isOutput=False)
    wy = nc.declare_dram_parameter("wy", [128, YOUT], FP16, isOutput=False)
    wx = nc.declare_dram_parameter("wx", [128, XOUT], FP16, isOutput=False)
    out = nc.declare_dram_parameter("out", [BC, ZSH, YOUT, XOUT], FP16, isOutput=True)
    outflat = out.rearrange("b z y x -> (b z y) x")  # [46080, 160]

    ztaps = _ztaps()

    with TileContext(nc) as tc:
        with (
            tc.tile_pool(name="const", bufs=1) as cpool,
            tc.tile_pool(name="io", bufs=1) as iopool,
            tc.tile_pool(name="l2", bufs=3) as l2pool,
            tc.tile_pool(name="stage", bufs=3) as stpool,
            tc.tile_pool(name="psy", bufs=2, space="PSUM") as psy,
            tc.tile_pool(name="psx", bufs=2, space="PSUM") as psx,
        ):
            L0 = iopool.tile([128, 6 * ZISH * XIN], FP16)
            nc.sync.dma_start(out=L0[:, :], in_=v[:, :])
            L0v = L0.rearrange("p (b z x) -> p b z x", b=6, z=ZISH)

            wyt = cpool.tile([128, YOUT], FP16)
            nc.sync.dma_start(out=wyt[:, :], in_=wy[:, :])
            wxt = cpool.tile([128, XOUT], FP16)
            nc.sync.dma_start(out=wxt[:, :], in_=wx[:, :])

            L1 = iopool.tile([128, 6 * ZSH * XIN], FP16)
            L1r = L1.rearrange("p (b k r x) -> p b k r x", b=6, k=5, r=4)
            L1z = L1.rearrange("p (b z x) -> p b z x", b=6, z=ZSH)

            # ---- z-pass on DVE: split by b-pairs so y(b0) can start early ----
            for bp in range(3):
                for r in range(4):
                    dst = L1r[0:116, 2 * bp:2 * bp + 2, :, r, :]
                    t0, w0 = ztaps[r][0]
                    nc.vector.tensor_scalar_mul(
                        dst, L0v[0:116, 2 * bp:2 * bp + 2, t0:t0 + 5, :], w0)
                    for t, w in ztaps[r][1:]:
                        nc.vector.scalar_tensor_tensor(
                            out=dst,
                            in0=L0v[0:116, 2 * bp:2 * bp + 2, t:t + 5, :],
                            scalar=w, in1=dst, op0=MULT, op1=ADD,
                        )

            # greedy copy-engine chooser balancing planned engine load (ns)
            load = {"v": 20600.0, "a": 1300.0}
            COST = {("v", "y"): 925.0, ("a", "y"): 825.0,
                    ("v", "x"): 1125.0, ("a", "x"): 985.0}

            def cp(kind, dst, src):
                eng = min("va", key=lambda e: load[e] + COST[(e, kind)])
                load[eng] += COST[(eng, kind)]
                if eng == "v":
                    nc.vector.tensor_copy(out=dst, in_=src)
                else:
                    nc.scalar.copy(dst, src)

            def emit_y(b, L2b):
                for zq in range(ZSH // 4):   # 4 zo per psum tile
                    py = psy.tile([128, 1024], FP32)
                    for h in range(2):
                        for i in range(2):
                            zo = 4 * zq + 2 * h + i
                            for g in range(2):
                                nc.tensor.matmul(
                                    py[64 * g:64 * g + XIN,
                                       512 * h + 192 * i:512 * h + 192 * (i + 1)],
                                    lhsT=L1z[64 * g:64 * g + YIN, b, zo, :],
                                    rhs=wyt[64 * g:64 * g + YIN, :],
                                    start=True, stop=True,
                                )
                    pyv = py.rearrange("p (h c) -> p h c", h=2)
                    dst = L2b[0:108, zq * 768:(zq + 1) * 768]
                    cp("y", dst.rearrange("p (h c) -> p h c", h=2),
                       pyv[0:108, :, 0:384])

            def emit_x(b, L2b):
                L2v = L2b.rearrange("p (m j) -> p m j", j=NJ)
                for g in range(2):
                    st = stpool.tile([128, NJ * XOUT], FP16)
                    stv = st.rearrange("p (s t c) -> p s t c", s=5, t=2)
                    for s in range(5):
                        px = psx.tile([128, 1024], FP32)
                        pxv = px.rearrange("p (t c) -> p t c", t=2)
                        for t in range(2):
                            for u in range(3):
                                j = 6 * s + 3 * t + u
                                nc.tensor.matmul(
                                    px[:, 512 * t + 160 * u:512 * t + 160 * (u + 1)],
                                    lhsT=L2v[64 * g:64 * g + XIN, :, j],
                                    rhs=wxt[64 * g:64 * g + XIN, :],
                                    start=True, stop=True,
                                )
                        cp("x", stv[:, s, :, :], pxv[:, :, 0:480])
                    base = (g * 6 + b) * M
                    for hp in range(2):   # split DMA for smoother drain
                        nc.sync.dma_start(
                            out=outflat[base + 1920 * hp:base + 1920 * (hp + 1), :]
                            .rearrange("(p r) x -> p (r x)", p=64),
                            in_=st[64 * hp:64 * (hp + 1), :],
                        )

            L2 = [None] * 6
            L2[0] = l2pool.tile([128, M], FP16, name='l2_0')
            emit_y(0, L2[0])
            for b in range(6):
                if b < 5:
                    L2[b + 1] = l2pool.tile([128, M], FP16, name=f'l2_{b+1}')
                    emit_y(b + 1, L2[b + 1])
                emit_x(b, L2[b])
    nc.compile()
    return nc


def _get_nc():
    if "nc" not in _NC_CACHE:
        _NC_CACHE["nc"] = _build_nc()
    return _NC_CACHE["nc"]


def _host_inputs(v):
    """Per-core input slabs + weight tiles."""
    v = np.asarray(v).astype(np.float32).reshape(BC, ZIN, YIN, XIN)

    wy128 = np.zeros((128, YOUT), dtype=np.float32)
    wy128[0:YIN] = _exp_mat(YIN, YOUT)
    wy128[64:64 + YIN] = wy128[0:YIN]
    wx128 = np.zeros((128, XOUT), dtype=np.float32)
    wx128[0:XIN] = _exp_mat(XIN, XOUT)
    wx128[64:64 + XIN] = wx128[0:XIN]
    wy_h = wy128.astype(np.float16)
    wx_h = wx128.astype(np.float16)

    in_maps = []
    for c in range(N_CORES):
        slab = np.zeros((128, 6, ZISH, XIN), dtype=np.float16)
        vv = v[:, 5 * c:5 * c + ZISH]                  # [12, 8, 52, 44]
        slab[0:YIN] = vv[0:6].transpose(2, 0, 1, 3)    # y b z x
        slab[64:64 + YIN] = vv[6:12].transpose(2, 0, 1, 3)
        in_maps.append({
            "v": slab.reshape(128, 6 * ZISH * XIN),
            "wy": wy_h, "wx": wx_h,
        })
    return in_maps


def kernel(v):
    from concourse.bass_utils import run_bass_kernel_spmd

    in_maps = _host_inputs(v)
    nc = _get_nc()
    res = run_bass_kernel_spmd(nc, in_maps, core_ids=list(range(N_CORES)))

    out = np.empty((BC, ZOUT, YOUT, XOUT), dtype=np.float32)
    for c in range(N_CORES):
        out[:, ZSH * c:ZSH * (c + 1)] = np.asarray(
            res.results[c]["out"]).astype(np.float32)
    return out.reshape(4, 3, ZOUT, YOUT, XOUT)


# revision 7
# speedup vs baseline: 1.5847x; 1.0871x over previous
"""Cubic B-spline FFD 3D upsampling kernel for Trainium2 (8 NeuronCores).

Reference: v [4,3,44,52,44] f32 -> out [4,3,160,192,160] f32 via three separable
stride-4 transposed convs (cubic B-spline, 15 taps) + crop.

Sharding: output z (160) split into 8 slabs of 20; core c reads input z-planes
[5c, 5c+8) (no halo) and writes out[:, :, 20c:20c+20].

Per-core pipeline (fp16 data, f32 PSUM):
  L0 [128=(g*64 + y52), (b6, zi8, xi44)]   one DMA in, host-packed layout
  z-pass DVE: 15 fused MACs over both g halves at once (partitions 0:116)
  L1 [128, (b6, zo20, xi44)]
  y-pass PE:  py[64g+xi44, zo2*192] = L1[y, b, zo, xi].T @ Wy   (4 mm / 2 zo)
  L2 [128, m=(zo20, yo192)=3840] per b  (copy downcast fp16, DVE/Act split)
  x-pass PE:  px[128=m-chunk(stride 30), xo160] = L2[xi, m].T @ Wx
  st [128, (r30, xo160)] fp16: partition p holds DRAM rows 30p..30p+29
  DMA out per (b,g): 9600B descriptors -> out fp16 [12, 20, 192, 160]
Host: cast fp16 -> f32, stack z-slabs.
"""

import numpy as np

N_CORES = 8
ZIN, YIN, XIN = 44, 52, 44
ZOUT, YOUT, XOUT = 160, 192, 160
BC = 12
ZSH = ZOUT // N_CORES      # 20 output z per core
ZISH = 8                   # input z planes per core
M = ZSH * YOUT             # 3840 rows per (b, g) block
NJ = 30                    # rows per partition in x-pass/st/DMA


def _bspline_kernel():
    x = (np.arange(15) - 7) / 4.0
    t = np.abs(x)
    return np.where(
        t < 1.0, 2.0 / 3.0 + (0.5 * t - 1.0) * t**2,
        np.where(t < 2.0, ((2.0 - t) ** 3) / 6.0, 0.0)
    ).astype(np.float32)


_W = _bspline_kernel()


def _exp_mat(n_in, n_out):
    """M[i, o] = weight of control point i on (post-crop) output o."""
    Mm = np.zeros((n_in, n_out), dtype=np.float32)
    for o in range(n_out):
        ilo = int(np.ceil((o - 3) / 4))
        ihi = (o + 11) // 4
        for i in range(max(ilo, 0), min(ihi, n_in - 1) + 1):
            n = 4 * i - o + 3
            if 0 <= n < 15:
                Mm[i, o] = _W[n]
    return Mm


def _ztaps():
    """Per phase r: list of (tap t, weight) with input plane = k + t for zo=4k+r."""
    out = []
    for r in range(4):
        taps = []
        for t in range(4):
            n = 4 * t + 3 - r
            if 0 <= n < 15:
                taps.append((t, float(_W[n])))
        out.append(taps)
    return out


_NC_CACHE = {}


def _build_nc():
    import concourse.bacc as bacc
    import concourse.mybir as mybir
    from concourse.tile import TileContext

    FP32 = mybir.dt.float32
    FP16 = mybir.dt.float16
    MULT = mybir.AluOpType.mult
    ADD = mybir.AluOpType.add

    nc = bacc.Bacc()
    v = nc.declare_dram_parameter("v", [128, 6 * ZISH * XIN], FP16, isOutput=False)
    wy = nc.declare_dram_parameter("wy", [128, YOUT], FP16, isOutput=False)
    wx = nc.declare_dram_parameter("wx", [128, XOUT], FP16, isOutput=False)
    out = nc.declare_dram_parameter("out", [BC, ZSH, YOUT, XOUT], FP16, isOutput=True)
    outflat = out.rearrange("b z y x -> (b z y) x")  # [46080, 160]

    ztaps = _ztaps()

    with TileContext(nc) as tc:
        with (
            tc.tile_pool(name="const", bufs=1) as cpool,
            tc.tile_pool(name="io", bufs=1) as iopool,
            tc.tile_pool(name="l2", bufs=4) as l2pool,
            tc.tile_pool(name="stage", bufs=4) as stpool,
            tc.tile_pool(name="psy", bufs=2, space="PSUM") as psy,
            tc.tile_pool(name="psx", bufs=3, space="PSUM") as psx,
        ):
            L0 = iopool.tile([128, 6 * ZISH * XIN], FP16)
            nc.sync.dma_start(out=L0[:, :], in_=v[:, :])
            L0v = L0.rearrange("p (b z x) -> p b z x", b=6, z=ZISH)

            wyt = cpool.tile([128, YOUT], FP16)
            nc.sync.dma_start(out=wyt[:, :], in_=wy[:, :])
            wxt = cpool.tile([128, XOUT], FP16)
            nc.sync.dma_start(out=wxt[:, :], in_=wx[:, :])

            L1 = iopool.tile([128, 6 * ZSH * XIN], FP16)
            L1r = L1.rearrange("p (b k r x) -> p b k r x", b=6, k=5, r=4)
            L1z = L1.rearrange("p (b z x) -> p b z x", b=6, z=ZSH)

            # ---- z-pass on DVE: split by b-pairs so y(b0) can start early ----
            for bp in range(3):
                for r in range(4):
                    dst = L1r[0:116, 2 * bp:2 * bp + 2, :, r, :]
                    t0, w0 = ztaps[r][0]
                    nc.vector.tensor_scalar_mul(
                        dst, L0v[0:116, 2 * bp:2 * bp + 2, t0:t0 + 5, :], w0)
                    for t, w in ztaps[r][1:]:
                        nc.vector.scalar_tensor_tensor(
                            out=dst,
                            in0=L0v[0:116, 2 * bp:2 * bp + 2, t:t + 5, :],
                            scalar=w, in1=dst, op0=MULT, op1=ADD,
                        )

            # greedy copy-engine chooser balancing planned engine load (ns)
            load = {"v": 20600.0, "a": 1300.0}
            COST = {("v", "y"): 525.0, ("a", "y"): 505.0,
                    ("v", "x"): 1125.0, ("a", "x"): 985.0}

            def cp(kind, dst, src):
                eng = min("va", key=lambda e: load[e] + COST[(e, kind)])
                load[eng] += COST[(eng, kind)]
                if eng == "v":
                    nc.vector.tensor_copy(out=dst, in_=src)
                else:
                    nc.scalar.copy(dst, src)

            def emit_y(b, L2b):
                for zp in range(ZSH // 2):   # 2 zo per psum tile (1 bank)
                    py = psy.tile([128, 512], FP32)
                    for i in range(2):
                        zo = 2 * zp + i
                        for g in range(2):
                            nc.tensor.matmul(
                                py[64 * g:64 * g + XIN,
                                   192 * i:192 * (i + 1)],
                                lhsT=L1z[64 * g:64 * g + YIN, b, zo, :],
                                rhs=wyt[64 * g:64 * g + YIN, :],
                                start=True, stop=True,
                            )
                    cp("y", L2b[0:108, zp * 384:(zp + 1) * 384],
                       py[0:108, 0:384])

            def emit_x(b, L2b):
                L2v = L2b.rearrange("p (m j) -> p m j", j=NJ)
                for g in range(2):
                    st = stpool.tile([128, NJ * XOUT], FP16)
                    stv = st.rearrange("p (s t c) -> p s t c", s=5, t=2)
                    for s in range(5):
                        px = psx.tile([128, 1024], FP32)
                        pxv = px.rearrange("p (t c) -> p t c", t=2)
                        for t in range(2):
                            for u in range(3):
                                j = 6 * s + 3 * t + u
                                nc.tensor.matmul(
                                    px[:, 512 * t + 160 * u:512 * t + 160 * (u + 1)],
                                    lhsT=L2v[64 * g:64 * g + XIN, :, j],
                                    rhs=wxt[64 * g:64 * g + XIN, :],
                                    start=True, stop=True,
                                )
                        cp("x", stv[:, s, :, :], pxv[:, :, 0:480])
                    base = (g * 6 + b) * M
                    for hp in range(2):   # split DMA for smoother drain
                        nc.sync.dma_start(
                            out=outflat[base + 1920 * hp:base + 1920 * (hp + 1), :]
                            .rearrange("(p r) x -> p (r x)", p=64),
                            in_=st[64 * hp:64 * (hp + 1), :],
                        )

            L2 = [None] * 6
            L2[0] = l2pool.tile([128, M], FP16, name='l2')
            emit_y(0, L2[0])
            for b in range(6):
                if b < 5:
                    L2[b + 1] = l2pool.tile([128, M], FP16, name='l2')
                    emit_y(b + 1, L2[b + 1])
                emit_x(b, L2[b])
    nc.compile()
    return nc


def _get_nc():
    if "nc" not in _NC_CACHE:
        _NC_CACHE["nc"] = _build_nc()
    return _NC_CACHE["nc"]


def _host_inputs(v):
    """Per-core input slabs + weight tiles."""
    v = np.asarray(v).astype(np.float32).reshape(BC, ZIN, YIN, XIN)

    wy128 = np.zeros((128, YOUT), dtype=np.float32)
    wy128[0:YIN] = _exp_mat(YIN, YOUT)
    wy128[64:64 + YIN] = wy128[0:YIN]
    wx128 = np.zeros((128, XOUT), dtype=np.float32)
    wx128[0:XIN] = _exp_mat(XIN, XOUT)
    wx128[64:64 + XIN] = wx128[0:XIN]
    wy_h = wy128.astype(np.float16)
    wx_h = wx128.astype(np.float16)

    in_maps = []
    for c in range(N_CORES):
        slab = np.zeros((128, 6, ZISH, XIN), dtype=np.float16)
        vv = v[:, 5 * c:5 * c + ZISH]                  # [12, 8, 52, 44]
        slab[0:YIN] = vv[0:6].transpose(2, 0, 1, 3)    # y b z x
        slab[64:64 + YIN] = vv[6:12].transpose(2, 0, 1, 3)
        in_maps.append({
            "v": slab.reshape(128, 6 * ZISH * XIN),
            "wy": wy_h, "wx": wx_h,
        })
    return in_maps


def kernel(v):
    from concourse.bass_utils import run_bass_kernel_spmd

    in_maps = _host_inputs(v)
    nc = _get_nc()
    res = run_bass_kernel_spmd(nc, in_maps, core_ids=list(range(N_CORES)))

    out = np.empty((BC, ZOUT, YOUT, XOUT), dtype=np.float32)
    for c in range(N_CORES):
        out[:, ZSH * c:ZSH * (c + 1)] = np.asarray(
            res.results[c]["out"]).astype(np.float32)
    return out.reshape(4, 3, ZOUT, YOUT, XOUT)


# revision 9
# speedup vs baseline: 1.8233x; 1.1506x over previous
"""Cubic B-spline FFD 3D upsampling kernel for Trainium2 (8 NeuronCores).

Reference: v [4,3,44,52,44] f32 -> out [4,3,160,192,160] f32 via three separable
stride-4 transposed convs (cubic B-spline, 15 taps) + crop.

Sharding: output z (160) split into 8 slabs of 20; core c reads input z-planes
[5c, 5c+8) (no halo) and writes out[:, :, 20c:20c+20].

Per-core pipeline (fp16 data, f32 PSUM):
  L0 [128=(g*64 + y52), (b6, zi8, xi44)]   one DMA in, host-packed layout
  z-pass DVE: 15 fused MACs over both g halves at once (partitions 0:116)
  L1 [128, (b6, zo20, xi44)]
  y-pass PE:  py[64g+xi44, zo2*192] = L1[y, b, zo, xi].T @ Wy   (4 mm / 2 zo)
  L2 [128, m=(zo20, yo192)=3840] per b  (copy downcast fp16, DVE/Act split)
  x-pass PE:  px[128=m-chunk(stride 30), xo160] = L2[xi, m].T @ Wx
  st [128, (r30, xo160)] fp16: partition p holds DRAM rows 30p..30p+29
  DMA out per (b,g): 9600B descriptors -> out fp16 [12, 20, 192, 160]
Host: cast fp16 -> f32, stack z-slabs.
"""

import numpy as np

N_CORES = 8
ZIN, YIN, XIN = 44, 52, 44
ZOUT, YOUT, XOUT = 160, 192, 160
BC = 12
ZSH = ZOUT // N_CORES      # 20 output z per core
ZISH = 8                   # input z planes per core
M = ZSH * YOUT             # 3840 rows per (b, g) block
NJ = 30                    # rows per partition in x-pass/st/DMA


def _bspline_kernel():
    x = (np.arange(15) - 7) / 4.0
    t = np.abs(x)
    return np.where(
        t < 1.0, 2.0 / 3.0 + (0.5 * t - 1.0) * t**2,
        np.where(t < 2.0, ((2.0 - t) ** 3) / 6.0, 0.0)
    ).astype(np.float32)


_W = _bspline_kernel()


def _exp_mat(n_in, n_out):
    """M[i, o] = weight of control point i on (post-crop) output o."""
    Mm = np.zeros((n_in, n_out), dtype=np.float32)
    for o in range(n_out):
        ilo = int(np.ceil((o - 3) / 4))
        ihi = (o + 11) // 4
        for i in range(max(ilo, 0), min(ihi, n_in - 1) + 1):
            n = 4 * i - o + 3
            if 0 <= n < 15:
                Mm[i, o] = _W[n]
    return Mm


def _ztaps():
    """Per phase r: list of (tap t, weight) with input plane = k + t for zo=4k+r."""
    out = []
    for r in range(4):
        taps = []
        for t in range(4):
            n = 4 * t + 3 - r
            if 0 <= n < 15:
                taps.append((t, float(_W[n])))
        out.append(taps)
    return out


_NC_CACHE = {}


def _build_nc():
    import concourse.bacc as bacc
    import concourse.mybir as mybir
    from concourse.tile import TileContext

    FP32 = mybir.dt.float32
    FP16 = mybir.dt.float16
    MULT = mybir.AluOpType.mult
    ADD = mybir.AluOpType.add

    nc = bacc.Bacc()
    v = nc.declare_dram_parameter("v", [128, 6 * ZISH * XIN], FP16, isOutput=False)
    wy = nc.declare_dram_parameter("wy", [128, YOUT], FP16, isOutput=False)
    wx = nc.declare_dram_parameter("wx", [128, XOUT], FP16, isOutput=False)
    out = nc.declare_dram_parameter("out", [BC, ZSH, YOUT, XOUT], FP16, isOutput=True)
    outflat = out.rearrange("b z y x -> (b z y) x")  # [46080, 160]

    ztaps = _ztaps()

    with TileContext(nc) as tc:
        with (
            tc.tile_pool(name="const", bufs=1) as cpool,
            tc.tile_pool(name="io", bufs=1) as iopool,
            tc.tile_pool(name="l2", bufs=4) as l2pool,
            tc.tile_pool(name="stage", bufs=4) as stpool,
            tc.tile_pool(name="psy", bufs=2, space="PSUM") as psy,
            tc.tile_pool(name="psx", bufs=3, space="PSUM") as psx,
        ):
            L0 = iopool.tile([128, 6 * ZISH * XIN], FP16)
            nc.sync.dma_start(out=L0[:, :], in_=v[:, :])
            L0v = L0.rearrange("p (b z x) -> p b z x", b=6, z=ZISH)

            wyt = cpool.tile([128, YOUT], FP16)
            nc.sync.dma_start(out=wyt[:, :], in_=wy[:, :])
            wxt = cpool.tile([128, XOUT], FP16)
            nc.sync.dma_start(out=wxt[:, :], in_=wx[:, :])

            L1 = iopool.tile([128, 6 * ZSH * XIN], FP16)
            L1r = L1.rearrange("p (b k r x) -> p b k r x", b=6, k=5, r=4)
            L1z = L1.rearrange("p (b z x) -> p b z x", b=6, z=ZSH)

            # ---- z-pass on DVE: chunks b0 | b1 | b23 | b45 for early y start ----
            for blo, bhi in ((0, 1), (1, 2), (2, 4), (4, 6)):
                for r in range(4):
                    dst = L1r[0:116, blo:bhi, :, r, :]
                    t0, w0 = ztaps[r][0]
                    nc.vector.tensor_scalar_mul(
                        dst, L0v[0:116, blo:bhi, t0:t0 + 5, :], w0)
                    for t, w in ztaps[r][1:]:
                        nc.vector.scalar_tensor_tensor(
                            out=dst,
                            in0=L0v[0:116, blo:bhi, t:t + 5, :],
                            scalar=w, in1=dst, op0=MULT, op1=ADD,
                        )

            # greedy copy-engine chooser balancing planned engine load (ns)
            load = {"v": 20600.0, "a": 1300.0}
            COST = {("v", "y"): 525.0, ("a", "y"): 505.0,
                    ("v", "x"): 1125.0, ("a", "x"): 985.0}

            def cp(kind, dst, src):
                eng = min("va", key=lambda e: load[e] + COST[(e, kind)])
                load[eng] += COST[(eng, kind)]
                if eng == "v":
                    nc.vector.tensor_copy(out=dst, in_=src)
                else:
                    nc.scalar.copy(dst, src)

            def emit_y(b, L2b):
                for zp in range(ZSH // 2):   # 2 zo per psum tile (1 bank)
                    py = psy.tile([128, 512], FP32)
                    for i in range(2):
                        zo = 2 * zp + i
                        for g in range(2):
                            nc.tensor.matmul(
                                py[64 * g:64 * g + XIN,
                                   192 * i:192 * (i + 1)],
                                lhsT=L1z[64 * g:64 * g + YIN, b, zo, :],
                                rhs=wyt[64 * g:64 * g + YIN, :],
                                start=True, stop=True,
                            )
                    cp("y", L2b[0:108, zp * 384:(zp + 1) * 384],
                       py[0:108, 0:384])

            def emit_x(b, L2b):
                L2v = L2b.rearrange("p (m j) -> p m j", j=NJ)
                for g in range(2):
                    st = stpool.tile([128, NJ * XOUT], FP16)
                    stv = st.rearrange("p (s t c) -> p s t c", s=5, t=2)
                    for s in range(5):
                        px = psx.tile([128, 1024], FP32)
                        pxv = px.rearrange("p (t c) -> p t c", t=2)
                        for t in range(2):
                            for u in range(3):
                                j = 6 * s + 3 * t + u
                                nc.tensor.matmul(
                                    px[:, 512 * t + 160 * u:512 * t + 160 * (u + 1)],
                                    lhsT=L2v[64 * g:64 * g + XIN, :, j],
                                    rhs=wxt[64 * g:64 * g + XIN, :],
                                    start=True, stop=True,
                                )
                        cp("x", stv[:, s, :, :], pxv[:, :, 0:480])
                        # DMA this s-slot now: rows 30p+6s..30p+6s+5
                        base = (g * 6 + b) * M
                        nc.sync.dma_start(
                            out=outflat[base:base + M, :]
                            .rearrange("(p w r) x -> p w r x", p=128, r=6)[:, s, :, :],
                            in_=st.rearrange("p (w r x) -> p w r x", w=5, r=6)[:, s, :, :],
                        )

            L2 = [None] * 6
            L2[0] = l2pool.tile([128, M], FP16, name='l2')
            emit_y(0, L2[0])
            for b in range(6):
                if b < 5:
                    L2[b + 1] = l2pool.tile([128, M], FP16, name='l2')
                    emit_y(b + 1, L2[b + 1])
                emit_x(b, L2[b])
    nc.compile()
    return nc


def _get_nc():
    if "nc" not in _NC_CACHE:
        _NC_CACHE["nc"] = _build_nc()
    return _NC_CACHE["nc"]


def _host_inputs(v):
    """Per-core input slabs + weight tiles."""
    v = np.asarray(v).astype(np.float32).reshape(BC, ZIN, YIN, XIN)

    wy128 = np.zeros((128, YOUT), dtype=np.float32)
    wy128[0:YIN] = _exp_mat(YIN, YOUT)
    wy128[64:64 + YIN] = wy128[0:YIN]
    wx128 = np.zeros((128, XOUT), dtype=np.float32)
    wx128[0:XIN] = _exp_mat(XIN, XOUT)
    wx128[64:64 + XIN] = wx128[0:XIN]
    wy_h = wy128.astype(np.float16)
    wx_h = wx128.astype(np.float16)

    in_maps = []
    for c in range(N_CORES):
        slab = np.zeros((128, 6, ZISH, XIN), dtype=np.float16)
        vv = v[:, 5 * c:5 * c + ZISH]                  # [12, 8, 52, 44]
        slab[0:YIN] = vv[0:6].transpose(2, 0, 1, 3)    # y b z x
        slab[64:64 + YIN] = vv[6:12].transpose(2, 0, 1, 3)
        in_maps.append({
            "v": slab.reshape(128, 6 * ZISH * XIN),
            "wy": wy_h, "wx": wx_h,
        })
    return in_maps


def kernel(v):
    from concourse.bass_utils import run_bass_kernel_spmd

    in_maps = _host_inputs(v)
    nc = _get_nc()
    res = run_bass_kernel_spmd(nc, in_maps, core_ids=list(range(N_CORES)))

    out = np.empty((BC, ZOUT, YOUT, XOUT), dtype=np.float32)
    for c in range(N_CORES):
        out[:, ZSH * c:ZSH * (c + 1)] = np.asarray(
            res.results[c]["out"]).astype(np.float32)
    return out.reshape(4, 3, ZOUT, YOUT, XOUT)
